# revision 14
# baseline (speedup 1.0000x reference)
"""Trainium2 Bass kernel for nn_Discriminator loss function.

Computes: MLP(3->64->64->3) scoring over (B, S-1) transition features,
Hermite-basis contraction, softmax over the batch dim, clamped
log-likelihood sum, mean over batch.

Sharding: the S-1=1023 output columns are split across 8 cores
(128 per core, core 7 padded by one duplicated column that the host
drops). The softmax runs over the batch dim, which is fully resident
per core, so no cross-core collectives are needed.

Per-core dataflow:
  1. DMA the (B, 129, 2) ys slice in 128-row chunks, PE-transpose into
     U buffers laid out (s*2+c on partitions, b on free). Row 2k = t_k,
     row 2k+1 = v_k, so each output column j's features (x0, t, xt) are
     the contiguous partition triple U[2j+1 : 2j+4].
  2. For each pair of columns (j, j+2) (packed block-diagonally, M=128)
     run the 3 matmuls in float32r with N=512 column tiles; relu1 on the
     scalar engine (bias fused), relu2 on the vector engine (bias+max
     fused). Layer-3 outputs of 4 pairs land in one PSUM bank at
     partition offsets 0/32/64/96 via tile_position col-tiling.
  3. Scatter the c coefficients into C (s on partitions, b on free),
     assemble p = c0 + c1*(2x) + c2*(4x^2-2) via a Horner pass with the
     Hermite change-of-basis folded into Wout, then do the batch-dim
     softmax + clamped log + row sums entirely with free-dim reductions.
Host sums the per-column partials and divides by B.
"""

import os
import sys

import numpy as np

_REPO = "/opt/trn_rl_repo"
if _REPO not in sys.path:
    sys.path.insert(0, _REPO)

import concourse.bass as bass
import concourse.tile as tile
from concourse import bacc
from concourse import mybir

AFT = mybir.ActivationFunctionType
ALU = mybir.AluOpType
F32 = mybir.dt.float32
F32R = mybir.dt.float32r
BF16 = mybir.dt.bfloat16

N_CORES = 8
LN_CLAMP = float(np.log(np.float64(1e-10)))

# Filled by kernel() after a traced run (test harness reads these).
LAST_RESULTS = None


def build_program(B, SL, bout_h, ntile=512):
    """Trace the per-core Bass program. SL = output columns per core.

    bout_h: the 3 Hermite-folded output-bias values (baked as immediates).
    """
    assert B % ntile == 0 and B % 128 == 0 and SL % 4 == 0
    nbch = B // 128          # 128-row batch chunks for the input transpose
    nbb = B // ntile         # N=512 column blocks per pair
    slp1 = SL + 1
    ru = 2 * slp1            # transposed rows: t/v interleaved per s
    b0p, b1p, b2p = (float(x) for x in bout_h)

    # U chunk starts every 120 rows so every pair slice [2j+1, 2j+8) and
    # every X row 2j+3 sits inside a single <=128-partition chunk.
    chunk_starts = [0]
    while chunk_starts[-1] + 128 < ru:
        chunk_starts.append(chunk_starts[-1] + 120)

    def chunk_for_pair(jb):
        for m, cs in enumerate(chunk_starts):
            w = min(128, ru - cs)
            if 2 * jb + 1 >= cs and 2 * jb + 8 <= cs + w:
                return m, cs
        raise AssertionError(f"no chunk for pair {jb}")

    def chunk_for_x(j):
        for m, cs in enumerate(chunk_starts):
            w = min(128, ru - cs)
            if 2 * j + 3 >= cs and 2 * j + 3 < cs + w:
                return m, cs
        raise AssertionError(f"no chunk for x row {j}")

    nc = bacc.Bacc("TRN2", target_bir_lowering=False, debug=False)
    ys = nc.declare_dram_parameter("ys", [B, ru], F32, isOutput=False)
    w1 = nc.declare_dram_parameter("w1blk", [7, 128], F32, isOutput=False)
    w2 = nc.declare_dram_parameter("w2blk", [128, 128], F32, isOutput=False)
    wo = nc.declare_dram_parameter("woblk", [128, 32], F32, isOutput=False)
    b1 = nc.declare_dram_parameter("b1c", [128, 1], F32, isOutput=False)
    b2 = nc.declare_dram_parameter("b2c", [128, 1], F32, isOutput=False)
    ident = nc.declare_dram_parameter("ident", [128, 128], F32, isOutput=False)
    outp = nc.declare_dram_parameter("out", [SL, 1], F32, isOutput=True)

    with tile.TileContext(nc) as tc:
        with (
            tc.tile_pool(name="consts", bufs=1) as consts,
            tc.tile_pool(name="big", bufs=1) as bigp,
        ):
            w1f = consts.tile([7, 128], F32)
            nc.sync.dma_start(w1f, w1[:])
            w2f = consts.tile([128, 128], F32)
            nc.sync.dma_start(w2f, w2[:])
            wof = consts.tile([128, 32], F32)
            nc.sync.dma_start(wof, wo[:])
            w1s = consts.tile([7, 128], F32R)
            nc.scalar.copy(w1s, w1f)
            w2s = consts.tile([128, 128], F32R)
            nc.scalar.copy(w2s, w2f)
            wos = consts.tile([128, 32], BF16)
            nc.scalar.copy(wos, wof)
            b1s = consts.tile([128, 1], F32)
            nc.sync.dma_start(b1s, b1[:])
            b2s = consts.tile([128, 1], F32)
            nc.sync.dma_start(b2s, b2[:])
            ids = consts.tile([128, 128], F32)
            nc.sync.dma_start(ids, ident[:])

            ubufs = []
            for m, cs in enumerate(chunk_starts):
                w = min(128, ru - cs)
                ubufs.append(bigp.tile([w, B], F32R, tag=f"U{m}", name=f"U{m}"))
            cb = bigp.tile([SL, 3, B], F32, tag="C")
            xb = bigp.tile([SL, B], F32, tag="X")
            pb = bigp.tile([SL, B], F32, tag="P")
            t1 = bigp.tile([SL, B], F32, tag="T1")
            t2 = bigp.tile([SL, B], F32, tag="T2")
            eb = bigp.tile([SL, B], F32, tag="E")
            mred = bigp.tile([SL, 1], F32, tag="m")
            negm = bigp.tile([SL, 1], F32, tag="nm")
            zred = bigp.tile([SL, 1], F32, tag="z")
            lz = bigp.tile([SL, 1], F32, tag="lz")
            mlz = bigp.tile([SL, 1], F32, tag="mlz")
            rs = bigp.tile([SL, 1], F32, tag="rs")

            # ---- phase 1: load + transpose into U ----
            with (
                tc.tile_pool(name="ld", bufs=3) as ldp,
                tc.tile_pool(name="tpps", bufs=3, space="PSUM") as tpp,
                tc.tile_pool(name="tppx", bufs=2, space="PSUM") as tpx,
            ):
                for i in range(nbch):
                    ld = ldp.tile([128, ru], F32, tag="ld")
                    nc.sync.dma_start(ld, ys[128 * i : 128 * (i + 1), :])
                    for m, cs in enumerate(chunk_starts):
                        w = min(128, ru - cs)
                        pt = tpp.tile([128, 128], F32, tag="tp")
                        nc.tensor.transpose(pt[:w, :], ld[:, cs : cs + w], ids)
                        dst = ubufs[m][:, 128 * i : 128 * (i + 1)]
                        nc.scalar.copy(dst, pt[:w, :])
                    # xt in (s, b) layout: transpose the strided v columns
                    # v_{j+1} = ld free-col 2*(j+1)+1 for j in [0, SL)
                    ptx = tpx.tile([128, 128], F32, tag="tpx")
                    nc.tensor.transpose(
                        ptx[:SL, :], ld[:, 3 : 2 + 2 * SL : 2], ids
                    )
                    nc.vector.tensor_copy(
                        xb[:, 128 * i : 128 * (i + 1)], ptx[:SL, :]
                    )

            # ---- phase 2: MLP over column pairs ----
            pair_bases = [4 * k + r for k in range(SL // 4) for r in (0, 1)]
            assert len(pair_bases) % 4 == 0
            quads = [pair_bases[4 * q : 4 * q + 4] for q in range(len(pair_bases) // 4)]

            with (
                tc.tile_pool(name="stage", bufs=6) as stagep,
                tc.tile_pool(name="hsb", bufs=3) as hp,
                tc.tile_pool(name="mmps", bufs=3, space="PSUM") as mmp,
                tc.tile_pool(name="cpps", bufs=2, space="PSUM") as cpp,
                tc.tile_pool(name="cstg", bufs=2) as cstp,
            ):
                for quad in quads:
                    stg = []
                    for jb in quad:
                        m, cs = chunk_for_pair(jb)
                        st = stagep.tile([7, B], F32R, tag="stg")
                        nc.sync.dma_start(
                            st, ubufs[m][2 * jb + 1 - cs : 2 * jb + 8 - cs, :]
                        )
                        stg.append(st)
                    for bb in range(nbb):
                        cpq = cpp.tile([128, ntile], F32, tag="cpq")
                        for pi, jb in enumerate(quad):
                            rhs1 = stg[pi][:, bb * ntile : (bb + 1) * ntile]
                            h1p = mmp.tile([128, ntile], F32, tag="h1p")
                            nc.tensor.matmul(
                                h1p, w1s, rhs1,
                                start=True, stop=True,
                            )
                            h1s = hp.tile([128, ntile], F32R, tag="h1s")
                            nc.scalar.activation(
                                h1s, h1p, AFT.Relu, bias=b1s, scale=1.0
                            )
                            h2p = mmp.tile([128, ntile], F32, tag="h2p")
                            nc.tensor.matmul(
                                h2p, w2s, h1s,
                                start=True, stop=True,
                            )
                            h2s = hp.tile([128, ntile], BF16, tag="h2s")
                            nc.vector.tensor_scalar(
                                out=h2s, in0=h2p, scalar1=b2s, scalar2=0.0,
                                op0=ALU.add, op1=ALU.max,
                            )
                            nc.tensor.matmul(
                                cpq[32 * pi : 32 * pi + 32, :],
                                wos, h2s,
                                start=True, stop=True,
                                tile_position=(0, 32 * pi),
                            )
                        cst = cstp.tile([128, ntile], F32, tag="cst")
                        nc.scalar.copy(cst, cpq)
                        for pi, jb in enumerate(quad):
                            for a in (0, 1):
                                src = cst[32 * pi + 3 * a : 32 * pi + 3 * a + 3, :]
                                j = jb + 2 * a
                                nc.sync.dma_start(
                                    cb[j : j + 1, :, bb * ntile : (bb + 1) * ntile],
                                    src,
                                )

            # ---- phase 3: assemble p, softmax over batch ----
            # p = (c2*x + c1)*x + c0  +  ((b2p*x + b1p)*x + b0p)
            nc.vector.tensor_mul(t1, cb[:, 2, :], xb)
            nc.vector.tensor_add(t1, t1, cb[:, 1, :])
            nc.vector.tensor_mul(t1, t1, xb)
            nc.vector.tensor_add(t1, t1, cb[:, 0, :])
            nc.vector.tensor_scalar(
                out=t2, in0=xb, scalar1=b2p, scalar2=b1p,
                op0=ALU.mult, op1=ALU.add,
            )
            nc.vector.tensor_mul(t2, t2, xb)
            nc.vector.tensor_scalar_add(t2, t2, b0p)
            nc.vector.tensor_add(pb, t1, t2)

            # softmax over batch (free dim) + clamped log-likelihood
            nc.vector.reduce_max(out=mred, in_=pb, axis=mybir.AxisListType.X)
            nc.vector.tensor_scalar_mul(negm, mred, -1.0)
            nc.scalar.activation(eb, pb, AFT.Exp, bias=negm, scale=1.0)
            nc.vector.reduce_sum(out=zred, in_=eb, axis=mybir.AxisListType.X)
            nc.scalar.activation(lz, zred, AFT.Ln)
            nc.vector.tensor_add(mlz, mred, lz)
            nc.vector.tensor_scalar(
                out=eb, in0=pb, scalar1=mlz, scalar2=LN_CLAMP,
                op0=ALU.subtract, op1=ALU.max,
            )
            nc.vector.reduce_sum(out=rs, in_=eb, axis=mybir.AxisListType.X)
            nc.sync.dma_start(outp[:], rs)

    nc.finalize()
    return nc


def make_host_inputs(ys_coeffs, W1, b1, W2, b2, Wout, bout, SL):
    """Build the per-core input maps and the Hermite-folded bias."""
    B, S, _ = ys_coeffs.shape
    f4 = np.float32
    n_cores = (S - 1 + SL - 1) // SL
    # pad S so every core can load SL+1 s-columns
    pad = n_cores * SL + 1 - S
    ys_pad = np.concatenate(
        [ys_coeffs] + [ys_coeffs[:, -1:, :]] * pad, axis=1
    ) if pad > 0 else ys_coeffs

    w1p = np.asarray(W1, f4)[[2, 1, 0], :]  # rows reordered to (x0, t, xt)
    w1blk = np.zeros((7, 128), f4)
    w1blk[0:3, 0:64] = w1p
    w1blk[4:7, 64:128] = w1p

    w2blk = np.zeros((128, 128), f4)
    w2blk[0:64, 0:64] = W2
    w2blk[64:128, 64:128] = W2

    # Hermite change of basis: herm = Mh @ [1, x, x^2]
    mh = np.array([[1, 0, 0], [0, 2, 0], [-2, 0, 4]], f4)
    wop = np.asarray(Wout, f4) @ mh
    woblk = np.zeros((128, 32), f4)
    woblk[0:64, 0:3] = wop
    woblk[64:128, 3:6] = wop
    bout_h = np.asarray(bout, f4) @ mh

    b1c = np.concatenate([b1, b1]).astype(f4)[:, None]
    b2c = np.concatenate([b2, b2]).astype(f4)[:, None]
    ident = np.eye(128, dtype=f4)

    in_maps = []
    for c in range(n_cores):
        sl = ys_pad[:, SL * c : SL * c + SL + 1, :].reshape(B, 2 * (SL + 1))
        in_maps.append({
            "ys": np.ascontiguousarray(sl, dtype=f4),
            "w1blk": w1blk,
            "w2blk": w2blk,
            "woblk": woblk,
            "b1c": b1c,
            "b2c": b2c,
            "ident": ident,
        })
    return in_maps, bout_h


def kernel(ys_coeffs, W1, b1, W2, b2, Wout, bout):
    global LAST_RESULTS
    from concourse.bass_utils import run_bass_kernel_spmd

    B, S, _ = ys_coeffs.shape
    SL = (S - 1 + N_CORES - 1) // N_CORES  # 128
    in_maps, bout_h = make_host_inputs(
        ys_coeffs, W1, b1, W2, b2, Wout, bout, SL
    )
    nc = build_program(B=B, SL=SL, bout_h=bout_h)
    trace = bool(os.environ.get("BASS_TRACE"))
    res = run_bass_kernel_spmd(
        nc, in_maps, list(range(N_CORES)), trace=trace
    )
    LAST_RESULTS = res
    lls = np.concatenate([res.results[c]["out"][:, 0] for c in range(N_CORES)])
    total = float(np.sum(lls[: S - 1].astype(np.float64)))
    return np.float32(total / B)


# revision 15
# speedup vs baseline: 1.5172x; 1.5172x over previous
"""Trainium2 Bass kernel for nn_Discriminator loss function.

Computes: MLP(3->64->64->3) scoring over (B, S-1) transition features,
Hermite-basis contraction, softmax over the batch dim, clamped
log-likelihood sum, mean over batch.

Sharding: the S-1=1023 output columns are split across 8 cores
(128 per core, core 7 padded by one duplicated column that the host
drops). The softmax runs over the batch dim, which is fully resident
per core, so no cross-core collectives are needed.

Per-core dataflow:
  1. DMA the (B, 129, 2) ys slice in 128-row chunks, PE-transpose into
     U buffers laid out (s*2+c on partitions, b on free). Row 2k = t_k,
     row 2k+1 = v_k, so each output column j's features (x0, t, xt) are
     the contiguous partition triple U[2j+1 : 2j+4].
  2. For each pair of columns (j, j+2) (packed block-diagonally, M=128)
     run the 3 matmuls in float32r with N=512 column tiles; relu1 on the
     scalar engine (bias fused), relu2 on the vector engine (bias+max
     fused). Layer-3 outputs of 4 pairs land in one PSUM bank at
     partition offsets 0/32/64/96 via tile_position col-tiling.
  3. Scatter the c coefficients into C (s on partitions, b on free),
     assemble p = c0 + c1*(2x) + c2*(4x^2-2) via a Horner pass with the
     Hermite change-of-basis folded into Wout, then do the batch-dim
     softmax + clamped log + row sums entirely with free-dim reductions.
Host sums the per-column partials and divides by B.
"""

import os
import sys

import numpy as np

_REPO = "/opt/trn_rl_repo"
if _REPO not in sys.path:
    sys.path.insert(0, _REPO)

import concourse.bass as bass
import concourse.tile as tile
from concourse import bacc
from concourse import mybir

AFT = mybir.ActivationFunctionType
ALU = mybir.AluOpType
F32 = mybir.dt.float32
F32R = mybir.dt.float32r
BF16 = mybir.dt.bfloat16

N_CORES = 8
LN_CLAMP = float(np.log(np.float64(1e-10)))

# Filled by kernel() after a traced run (test harness reads these).
LAST_RESULTS = None


def build_program(B, SL, bout_h, ntile=512):
    """Trace the per-core Bass program. SL = output columns per core.

    bout_h: the 3 Hermite-folded output-bias values (baked as immediates).
    """
    assert B % ntile == 0 and B % 128 == 0 and SL % 4 == 0
    nbch = B // 128          # 128-row batch chunks for the input transpose
    nbb = B // ntile         # N=512 column blocks per pair
    slp1 = SL + 1
    ru = 2 * slp1            # transposed rows: t/v interleaved per s
    b0p, b1p, b2p = (float(x) for x in bout_h)

    # U chunk starts every 120 rows so every pair slice [2j+1, 2j+8) and
    # every X row 2j+3 sits inside a single <=128-partition chunk.
    chunk_starts = [0]
    while chunk_starts[-1] + 128 < ru:
        chunk_starts.append(chunk_starts[-1] + 120)

    def chunk_for_pair(jb):
        for m, cs in enumerate(chunk_starts):
            w = min(128, ru - cs)
            if 2 * jb + 1 >= cs and 2 * jb + 8 <= cs + w:
                return m, cs
        raise AssertionError(f"no chunk for pair {jb}")

    def chunk_for_x(j):
        for m, cs in enumerate(chunk_starts):
            w = min(128, ru - cs)
            if 2 * j + 3 >= cs and 2 * j + 3 < cs + w:
                return m, cs
        raise AssertionError(f"no chunk for x row {j}")

    nc = bacc.Bacc("TRN2", target_bir_lowering=False, debug=False)
    ys = nc.declare_dram_parameter("ys", [B, ru], F32, isOutput=False)
    w1 = nc.declare_dram_parameter("w1blk", [7, 128], F32, isOutput=False)
    w2 = nc.declare_dram_parameter("w2blk", [128, 128], F32, isOutput=False)
    wo = nc.declare_dram_parameter("woblk", [128, 32], F32, isOutput=False)
    b1 = nc.declare_dram_parameter("b1c", [128, 1], F32, isOutput=False)
    b2 = nc.declare_dram_parameter("b2c", [128, 1], F32, isOutput=False)
    ident = nc.declare_dram_parameter("ident", [128, 128], F32, isOutput=False)
    outp = nc.declare_dram_parameter("out", [SL, 1], F32, isOutput=True)

    with tile.TileContext(nc) as tc:
        with (
            tc.tile_pool(name="consts", bufs=1) as consts,
            tc.tile_pool(name="big", bufs=1) as bigp,
        ):
            w1f = consts.tile([7, 128], F32)
            nc.sync.dma_start(w1f, w1[:])
            w2f = consts.tile([128, 128], F32)
            nc.sync.dma_start(w2f, w2[:])
            wof = consts.tile([128, 32], F32)
            nc.sync.dma_start(wof, wo[:])
            w1s = consts.tile([7, 128], BF16)
            nc.scalar.copy(w1s, w1f)
            w2s = consts.tile([128, 128], BF16)
            nc.scalar.copy(w2s, w2f)
            wos = consts.tile([128, 32], BF16)
            nc.scalar.copy(wos, wof)
            b1s = consts.tile([128, 1], F32)
            nc.sync.dma_start(b1s, b1[:])
            b2s = consts.tile([128, 1], F32)
            nc.sync.dma_start(b2s, b2[:])
            ids = consts.tile([128, 128], F32)
            nc.sync.dma_start(ids, ident[:])

            ubufs = []
            for m, cs in enumerate(chunk_starts):
                w = min(128, ru - cs)
                ubufs.append(bigp.tile([w, B], BF16, tag=f"U{m}", name=f"U{m}"))
            cb = bigp.tile([SL, 3, B], F32, tag="C")
            xb = bigp.tile([SL, B], F32, tag="X")
            pb = bigp.tile([SL, B], F32, tag="P")
            t1 = bigp.tile([SL, B], F32, tag="T1")
            t2 = bigp.tile([SL, B], F32, tag="T2")
            eb = bigp.tile([SL, B], F32, tag="E")
            mred = bigp.tile([SL, 1], F32, tag="m")
            negm = bigp.tile([SL, 1], F32, tag="nm")
            zred = bigp.tile([SL, 1], F32, tag="z")
            lz = bigp.tile([SL, 1], F32, tag="lz")
            mlz = bigp.tile([SL, 1], F32, tag="mlz")
            rs = bigp.tile([SL, 1], F32, tag="rs")

            # ---- phase 1: load + transpose into U ----
            with (
                tc.tile_pool(name="ld", bufs=3) as ldp,
                tc.tile_pool(name="tpps", bufs=3, space="PSUM") as tpp,
                tc.tile_pool(name="tppx", bufs=2, space="PSUM") as tpx,
            ):
                for i in range(nbch):
                    ld = ldp.tile([128, ru], F32, tag="ld")
                    nc.sync.dma_start(ld, ys[128 * i : 128 * (i + 1), :])
                    for m, cs in enumerate(chunk_starts):
                        w = min(128, ru - cs)
                        pt = tpp.tile([128, 128], F32, tag="tp")
                        nc.tensor.transpose(pt[:w, :], ld[:, cs : cs + w], ids)
                        dst = ubufs[m][:, 128 * i : 128 * (i + 1)]
                        nc.scalar.copy(dst, pt[:w, :])
                    # xt in (s, b) layout: transpose the strided v columns
                    # v_{j+1} = ld free-col 2*(j+1)+1 for j in [0, SL)
                    ptx = tpx.tile([128, 128], F32, tag="tpx")
                    nc.tensor.transpose(
                        ptx[:SL, :], ld[:, 3 : 2 + 2 * SL : 2], ids
                    )
                    nc.vector.tensor_copy(
                        xb[:, 128 * i : 128 * (i + 1)], ptx[:SL, :]
                    )

            # ---- phase 2: MLP over column pairs (software-pipelined) ----
            pair_bases = [4 * k + r for k in range(SL // 4) for r in (0, 1)]
            assert len(pair_bases) % 4 == 0
            quads = [pair_bases[4 * q : 4 * q + 4] for q in range(len(pair_bases) // 4)]

            # flat supertile list: (quad, bb, pi); the PE stream is emitted
            # with a 2-beat skew between layers so relu latencies are hidden
            sts = [
                (qi, bb, pi)
                for qi in range(len(quads))
                for bb in range(nbb)
                for pi in range(4)
            ]
            nst = len(sts)

            with (
                tc.tile_pool(name="stage", bufs=9) as stagep,
                tc.tile_pool(name="hsb", bufs=3) as hp,
                tc.tile_pool(name="mmps", bufs=3, space="PSUM") as mmp,
                tc.tile_pool(name="cpps", bufs=2, space="PSUM") as cpp,
                tc.tile_pool(name="cstg", bufs=2) as cstp,
            ):
                stg = {}          # (qi, pi) -> stage tile
                h1p_t = {}
                h1s_t = {}
                h2p_t = {}
                h2s_t = {}
                cpq_t = {}        # (qi, bb) -> psum tile

                def load_stage(qi):
                    for pi, jb in enumerate(quads[qi]):
                        m, cs = chunk_for_pair(jb)
                        st = stagep.tile([7, B], BF16, tag="stg", name="stg")
                        nc.sync.dma_start(
                            st, ubufs[m][2 * jb + 1 - cs : 2 * jb + 8 - cs, :]
                        )
                        stg[(qi, pi)] = st

                load_stage(0)
                if len(quads) > 1:
                    load_stage(1)

                for t in range(nst + 4):
                    # stage A: layer-1 matmul for beat t
                    if t < nst:
                        qi, bb, pi = sts[t]
                        if pi == 0 and bb == 0 and qi + 2 < len(quads):
                            load_stage(qi + 2)
                        rhs1 = stg[(qi, pi)][:, bb * ntile : (bb + 1) * ntile]
                        h1p = mmp.tile([128, ntile], F32, tag="h1p", name="h1p")
                        nc.tensor.matmul(h1p, w1s, rhs1, start=True, stop=True)
                        h1p_t[t] = h1p
                    # stage B: relu1 for beat t-1
                    if 0 <= t - 1 < nst:
                        h1s = hp.tile([128, ntile], BF16, tag="h1s", name="h1s")
                        if t % 2 == 0:
                            nc.scalar.activation(
                                h1s, h1p_t[t - 1], AFT.Relu, bias=b1s, scale=1.0
                            )
                        else:
                            nc.vector.tensor_scalar(
                                out=h1s, in0=h1p_t[t - 1], scalar1=b1s,
                                scalar2=0.0, op0=ALU.add, op1=ALU.max,
                            )
                        del h1p_t[t - 1]
                        h1s_t[t - 1] = h1s
                    # stage C: layer-2 matmul for beat t-2
                    if 0 <= t - 2 < nst:
                        h2p = mmp.tile([128, ntile], F32, tag="h2p", name="h2p")
                        nc.tensor.matmul(
                            h2p, w2s, h1s_t[t - 2], start=True, stop=True
                        )
                        del h1s_t[t - 2]
                        h2p_t[t - 2] = h2p
                    # stage D: relu2 for beat t-3
                    if 0 <= t - 3 < nst:
                        h2s = hp.tile([128, ntile], BF16, tag="h2s", name="h2s")
                        if t % 2 == 0:
                            nc.vector.tensor_scalar(
                                out=h2s, in0=h2p_t[t - 3], scalar1=b2s,
                                scalar2=0.0, op0=ALU.add, op1=ALU.max,
                            )
                        else:
                            nc.scalar.activation(
                                h2s, h2p_t[t - 3], AFT.Relu, bias=b2s, scale=1.0
                            )
                        del h2p_t[t - 3]
                        h2s_t[t - 3] = h2s
                    # stage E: layer-3 matmul for beat t-4, packed by pair into
                    # one PSUM bank via tile_position col-tiling
                    if 0 <= t - 4 < nst:
                        qi, bb, pi = sts[t - 4]
                        if pi == 0:
                            cpq_t[(qi, bb)] = cpp.tile(
                                [128, ntile], F32, tag="cpq", name="cpq"
                            )
                        cpq = cpq_t[(qi, bb)]
                        nc.tensor.matmul(
                            cpq[32 * pi : 32 * pi + 32, :],
                            wos, h2s_t[t - 4],
                            start=True, stop=True,
                            tile_position=(0, 32 * pi),
                        )
                        del h2s_t[t - 4]
                        if pi == 3:
                            cst = cstp.tile([128, ntile], F32, tag="cst", name="cst")
                            if (qi * nbb + bb) % 2 == 0:
                                nc.scalar.copy(cst, cpq)
                            else:
                                nc.vector.tensor_copy(cst, cpq)
                            del cpq_t[(qi, bb)]
                            for pj, jb in enumerate(quads[qi]):
                                for a in (0, 1):
                                    srcp = cst[32 * pj + 3 * a : 32 * pj + 3 * a + 3, :]
                                    j = jb + 2 * a
                                    nc.sync.dma_start(
                                        cb[j : j + 1, :, bb * ntile : (bb + 1) * ntile],
                                        srcp,
                                    )

            # ---- phase 3: assemble p, softmax over batch ----
            # p = (c2*x + c1)*x + c0  +  ((b2p*x + b1p)*x + b0p)
            nc.vector.tensor_mul(t1, cb[:, 2, :], xb)
            nc.vector.tensor_add(t1, t1, cb[:, 1, :])
            nc.vector.tensor_mul(t1, t1, xb)
            nc.vector.tensor_add(t1, t1, cb[:, 0, :])
            nc.vector.tensor_scalar(
                out=t2, in0=xb, scalar1=b2p, scalar2=b1p,
                op0=ALU.mult, op1=ALU.add,
            )
            nc.vector.tensor_mul(t2, t2, xb)
            nc.vector.tensor_scalar_add(t2, t2, b0p)
            nc.vector.tensor_add(pb, t1, t2)

            # softmax over batch (free dim) + clamped log-likelihood
            nc.vector.reduce_max(out=mred, in_=pb, axis=mybir.AxisListType.X)
            nc.vector.tensor_scalar_mul(negm, mred, -1.0)
            nc.scalar.activation(eb, pb, AFT.Exp, bias=negm, scale=1.0)
            nc.vector.reduce_sum(out=zred, in_=eb, axis=mybir.AxisListType.X)
            nc.scalar.activation(lz, zred, AFT.Ln)
            nc.vector.tensor_add(mlz, mred, lz)
            nc.vector.tensor_scalar(
                out=eb, in0=pb, scalar1=mlz, scalar2=LN_CLAMP,
                op0=ALU.subtract, op1=ALU.max,
            )
            nc.vector.reduce_sum(out=rs, in_=eb, axis=mybir.AxisListType.X)
            nc.sync.dma_start(outp[:], rs)

    nc.finalize()
    return nc


def make_host_inputs(ys_coeffs, W1, b1, W2, b2, Wout, bout, SL):
    """Build the per-core input maps and the Hermite-folded bias."""
    B, S, _ = ys_coeffs.shape
    f4 = np.float32
    n_cores = (S - 1 + SL - 1) // SL
    # pad S so every core can load SL+1 s-columns
    pad = n_cores * SL + 1 - S
    ys_pad = np.concatenate(
        [ys_coeffs] + [ys_coeffs[:, -1:, :]] * pad, axis=1
    ) if pad > 0 else ys_coeffs

    w1p = np.asarray(W1, f4)[[2, 1, 0], :]  # rows reordered to (x0, t, xt)
    w1blk = np.zeros((7, 128), f4)
    w1blk[0:3, 0:64] = w1p
    w1blk[4:7, 64:128] = w1p

    w2blk = np.zeros((128, 128), f4)
    w2blk[0:64, 0:64] = W2
    w2blk[64:128, 64:128] = W2

    # Hermite change of basis: herm = Mh @ [1, x, x^2]
    mh = np.array([[1, 0, 0], [0, 2, 0], [-2, 0, 4]], f4)
    wop = np.asarray(Wout, f4) @ mh
    woblk = np.zeros((128, 32), f4)
    woblk[0:64, 0:3] = wop
    woblk[64:128, 3:6] = wop
    bout_h = np.asarray(bout, f4) @ mh

    b1c = np.concatenate([b1, b1]).astype(f4)[:, None]
    b2c = np.concatenate([b2, b2]).astype(f4)[:, None]
    ident = np.eye(128, dtype=f4)

    in_maps = []
    for c in range(n_cores):
        sl = ys_pad[:, SL * c : SL * c + SL + 1, :].reshape(B, 2 * (SL + 1))
        in_maps.append({
            "ys": np.ascontiguousarray(sl, dtype=f4),
            "w1blk": w1blk,
            "w2blk": w2blk,
            "woblk": woblk,
            "b1c": b1c,
            "b2c": b2c,
            "ident": ident,
        })
    return in_maps, bout_h


def kernel(ys_coeffs, W1, b1, W2, b2, Wout, bout):
    global LAST_RESULTS
    from concourse.bass_utils import run_bass_kernel_spmd

    B, S, _ = ys_coeffs.shape
    SL = (S - 1 + N_CORES - 1) // N_CORES  # 128
    in_maps, bout_h = make_host_inputs(
        ys_coeffs, W1, b1, W2, b2, Wout, bout, SL
    )
    nc = build_program(B=B, SL=SL, bout_h=bout_h)
    trace = bool(os.environ.get("BASS_TRACE"))
    res = run_bass_kernel_spmd(
        nc, in_maps, list(range(N_CORES)), trace=trace
    )
    LAST_RESULTS = res
    lls = np.concatenate([res.results[c]["out"][:, 0] for c in range(N_CORES)])
    total = float(np.sum(lls[: S - 1].astype(np.float64)))
    return np.float32(total / B)
